# revision 12
# baseline (speedup 1.0000x reference)
"""DeepReservoir (leaky ESN, 4 modules) Trainium2 Bass kernel.

Problem: h[t] = (1-a)*h[t-1] + a*tanh(u[t] @ Kin + h[t-1] @ W + bias) per
module, T=8192 steps, U=1024 units, a=0.9, batch 1.  Output = all states,
modules concatenated on the feature axis: [1, T, 4*1024].

Strategy (module parallel x 2-way time split):
  - One reservoir module per NeuronCore pair: module m runs on core m
    (first half of the time axis) and core m+4 (second half).  The leaky
    ESN map is strongly contracting (leak 0.1 + saturated tanh): a
    trajectory restarted from h=0 converges to the true one within ~16
    steps (measured <=1e-4 rel).  Core m+4 therefore starts BURN=128
    steps before the midpoint from h=0 and its first BURN outputs are
    discarded -> per-core serial steps drop from T to T/2+BURN (1.9x).
  - The input projection c[t] = u[t] @ Kin + bias has no time dependence
    and is tiny; computed on the host, shipped pre-swizzled into the
    exact per-chunk SBUF layout so the per-iteration DMA is one fully
    contiguous block.
  - The time scan is the serial bottleneck: per step a [1024]x[1024,1024]
    matvec on TensorE as 64 LDWEIGHTS+MATMUL pairs of [128,128]x[128,1].
    Measured on HW: the pair cost (~41 ns) is pure NX instruction-issue
    overhead, independent of weight dtype (bf16 == fp8), so the step
    floor is 64 pairs x 41 ns = 2.6 us and weights stay bf16 for
    accuracy.  Leaky a is folded in: W' = a*W; state kept fp32 via the
    rescaled recurrence h'[t] = (1-a)*h'[t-1] + tanh(W' h'[t-1] + c[t]);
    the output is a*h'.
  - Per step the matmuls are phase-ordered (contraction tiles 0-3 for all
    output tiles, then finish output tiles 0-3, then 4-7) so ScalarE/
    VectorE process the first half of the new state while TensorE
    finishes the second half, and the next step's matmuls (which need
    only the first half as contraction input) start immediately.
  - tanh on ScalarE, z+c add and leaky blends on VectorE; the bf16 copy
    of the new state is written first to unblock TensorE.
  - Output states are staged in SBUF and DMAd per 32-step chunk in the
    SBUF-native layout; the host inverts the layout after gathering.
"""

import numpy as np
import ml_dtypes

import concourse.bacc as bacc
import concourse.tile as tile
import concourse.mybir as mybir
from concourse.bass import ds
from concourse.bass_utils import run_bass_kernel_spmd

F32 = mybir.dt.float32
BF16 = mybir.dt.bfloat16

UNITS = 1024
IN = 64
KT = 8  # contraction tiles (1024/128)
MT = 8  # output-unit tiles (1024/128)
P = 128

LEAKY = np.float32(0.9)
ONE_MINUS_LEAKY = float(np.float32(1.0) - np.float32(0.9))

N_CORES = 8
N_MODULES = 4
BURN = 128  # burn-in steps for the second-half cores


def build_nc(T_local: int, unroll: int):
    """Build the single-core SPMD Bass program for one reservoir module
    scanning T_local steps."""
    assert T_local % unroll == 0 and unroll % 2 == 0
    nchunk = T_local // unroll
    nc = bacc.Bacc("TRN2", debug=False)

    wT = nc.dram_tensor("wT", [UNITS, UNITS], BF16, kind="ExternalInput")
    # c pre-swizzled on host: c_in[chunk, p, s, j] = c[chunk*unroll+s, j*128+p]
    c_in = nc.dram_tensor("c_in", [nchunk, P, unroll, MT], F32, kind="ExternalInput")
    # output in SBUF-native layout: hs[chunk, p, s, j] = h[chunk*unroll+s, j*128+p]
    # bf16: halves output DMA + host transfer; rounding ~0.2% RMS, well
    # within the error budget
    hs = nc.dram_tensor("hs", [nchunk, P, unroll, MT], F32, kind="ExternalOutput")

    with tile.TileContext(nc) as tc:
        with (
            tc.tile_pool(name="const", bufs=1) as const_pool,
            tc.tile_pool(name="cin", bufs=2) as cin_pool,
            tc.tile_pool(name="hout", bufs=2) as hout_pool,
            tc.tile_pool(name="work", bufs=2) as work_pool,
            tc.tile_pool(name="zpsum", bufs=2, space="PSUM") as zpsum_pool,
        ):
            # weights: w_sb[p, k, m, c] = W'[k*128+p, m*128+c]
            w_sb = const_pool.tile([P, KT, MT, P], BF16)
            nc.sync.dma_start(
                w_sb[:], wT[:, :].rearrange("(k p) (m c) -> p k m c", p=P, c=P)
            )

            # persistent scan state (ping-pong on dim 1 by step parity)
            hstate = const_pool.tile([P, 2, MT], F32)  # h' fp32 master
            h16 = const_pool.tile([P, 2, MT], BF16)  # bf16 copy for PE rhs
            nc.vector.memset(hstate[:, 1, :], 0.0)
            nc.vector.memset(h16[:, 1, :], 0.0)

            c_v = c_in[:, :, :, :].rearrange("c p s j -> p c s j")
            hs_v = hs[:, :, :, :].rearrange("c p s j -> p c s j")

            with tc.For_i(
                0,
                nchunk,
                1,
                hint_engines=(mybir.EngineType.PE, mybir.EngineType.Activation),
            ) as iv:
                cchunk = cin_pool.tile([P, unroll, MT], F32, tag="cchunk")
                nc.sync.dma_start(cchunk[:], c_v[:, ds(iv, 1), :, :])
                hstage = hout_pool.tile([P, unroll, MT], F32, tag="hstage")

                for s in range(unroll):
                    cur = s % 2
                    prev = 1 - cur
                    zA = zpsum_pool.tile([P, 4], F32, tag="zA")
                    zB = zpsum_pool.tile([P, 4], F32, tag="zB")

                    def mm(k, m, start, stop):
                        zt = zA if m < 4 else zB
                        nc.tensor.matmul(
                            zt[:, (m % 4) : (m % 4) + 1],
                            w_sb[:, k, m, :],
                            h16[:, prev, k : k + 1],
                            start=start,
                            stop=stop,
                        )

                    # phase 1: contraction tiles 0-3 (only needs half A of
                    # h16, which the previous step produced early)
                    for k in range(4):
                        for m in range(MT):
                            mm(k, m, start=(k == 0 and m % 4 == 0), stop=False)
                    # phase 2a: finish z columns 0-3 so ScalarE can start
                    for m in range(4):
                        for k in range(4, 8):
                            mm(k, m, start=False, stop=(k == 7 and m == 3))
                    # phase 2b: finish z columns 4-7
                    for m in range(4, 8):
                        for k in range(4, 8):
                            mm(k, m, start=False, stop=(k == 7 and m == 7))

                    zc = work_pool.tile([P, MT], F32, tag="zc")
                    o32 = work_pool.tile([P, MT], F32, tag="o32")
                    for (lo, hi), zt in (((0, 4), zA), ((4, 8), zB)):
                        # zc = z + c[t]
                        nc.vector.tensor_add(
                            zc[:, lo:hi], zt[:, 0:4], cchunk[:, s, lo:hi]
                        )
                        # o = tanh(zc)
                        nc.scalar.activation(
                            o32[:, lo:hi],
                            zc[:, lo:hi],
                            mybir.ActivationFunctionType.Tanh,
                        )
                        # critical-path first: bf16 state for the next matmuls
                        nc.vector.scalar_tensor_tensor(
                            out=h16[:, cur, lo:hi],
                            in0=hstate[:, prev, lo:hi],
                            scalar=ONE_MINUS_LEAKY,
                            in1=o32[:, lo:hi],
                            op0=mybir.AluOpType.mult,
                            op1=mybir.AluOpType.add,
                        )
                        # fp32 master state (off critical path)
                        nc.vector.scalar_tensor_tensor(
                            out=hstate[:, cur, lo:hi],
                            in0=hstate[:, prev, lo:hi],
                            scalar=ONE_MINUS_LEAKY,
                            in1=o32[:, lo:hi],
                            op0=mybir.AluOpType.mult,
                            op1=mybir.AluOpType.add,
                        )
                    # output h[t] = a * h'[t]
                    nc.vector.tensor_scalar_mul(
                        hstage[:, s, :], hstate[:, cur, :], float(LEAKY)
                    )

                nc.sync.dma_start(hs_v[:, ds(iv, 1), :, :], hstage[:])

    nc.compile()
    return nc


def _t_local(T: int, unroll: int) -> int:
    assert T % 2 == 0
    t = T // 2 + BURN
    # round up to a whole number of chunks
    t = ((t + unroll - 1) // unroll) * unroll
    return t


def _prep_in_maps(u, kernel, rec_kernel, bias, T, unroll):
    """Per-core inputs: core c runs module c%4; cores 0-3 scan the window
    [0, T_local), cores 4-7 the window [T - T_local, T)."""
    T_local = _t_local(T, unroll)
    nchunk = T_local // unroll
    u0 = np.asarray(u[0], dtype=np.float32)  # [T, 64]
    in_maps = []
    for core in range(N_CORES):
        m = core % N_MODULES
        half = core // N_MODULES
        t0 = 0 if half == 0 else T - T_local
        wT = np.ascontiguousarray(
            (np.asarray(rec_kernel[m], dtype=np.float32) * LEAKY).astype(
                ml_dtypes.bfloat16
            )
        )
        # c[t, u] = u[t] @ Kin + bias  (fp32, host)
        c = u0[t0 : t0 + T_local] @ np.asarray(kernel[m], dtype=np.float32) + (
            np.asarray(bias[m], dtype=np.float32)
        )
        # -> c_in[chunk, p, s, j]
        c_sw = np.ascontiguousarray(
            c.reshape(nchunk, unroll, MT, P).transpose(0, 3, 1, 2)
        )
        in_maps.append({"wT": wT, "c_in": c_sw})
    return in_maps


def _unswizzle(hs_dev, T_local, unroll):
    # hs_dev[chunk, p, s, j] -> [T_local, 1024] with unit u = j*128+p
    return np.ascontiguousarray(
        hs_dev.astype(np.float32).transpose(0, 2, 3, 1).reshape(T_local, UNITS)
    )


def _assemble(per_core_hs, T, unroll):
    """per_core_hs: list of 8 arrays [nchunk, P, unroll, MT] -> [1, T, 4096]."""
    T_local = _t_local(T, unroll)
    outs = []
    for m in range(N_MODULES):
        first = _unswizzle(per_core_hs[m], T_local, unroll)[: T // 2]
        second = _unswizzle(per_core_hs[m + N_MODULES], T_local, unroll)[
            T_local - (T - T // 2) :
        ]
        outs.append(np.concatenate([first, second], axis=0))
    out = np.concatenate(outs, axis=1)  # [T, 4096]
    return out[None].astype(np.float32)


_NC_CACHE = {}


def run(u, kernel, rec_kernel, bias, unroll=64, trace=False):
    T = u.shape[1]
    T_local = _t_local(T, unroll)
    key = (T_local, unroll)
    if key not in _NC_CACHE:
        _NC_CACHE[key] = build_nc(T_local, unroll)
    nc = _NC_CACHE[key]
    in_maps = _prep_in_maps(u, kernel, rec_kernel, bias, T, unroll)
    res = run_bass_kernel_spmd(
        nc, in_maps, core_ids=list(range(N_CORES)), trace=trace
    )
    out = _assemble([res.results[c]["hs"] for c in range(N_CORES)], T, unroll)
    return out, res


def kernel(u, kernel, rec_kernel, bias):
    out, _ = run(u, kernel, rec_kernel, bias)
    return out


# revision 15
# speedup vs baseline: 1.0070x; 1.0070x over previous
"""DeepReservoir (leaky ESN, 4 modules) Trainium2 Bass kernel.

Problem: h[t] = (1-a)*h[t-1] + a*tanh(u[t] @ Kin + h[t-1] @ W + bias) per
module, T=8192 steps, U=1024 units, a=0.9, batch 1.  Output = all states,
modules concatenated on the feature axis: [1, T, 4*1024].

Strategy (module parallel x 2-way time split):
  - One reservoir module per NeuronCore pair: module m runs on core m
    (first half of the time axis) and core m+4 (second half).  The leaky
    ESN map is strongly contracting (leak 0.1 + saturated tanh): a
    trajectory restarted from h=0 converges to the true one within ~16
    steps (measured <=1e-4 rel).  Core m+4 therefore starts BURN steps
    before the midpoint from h=0 and its first BURN outputs are
    discarded -> per-core serial steps drop from T to T/2+BURN (1.9x).
  - The input projection c[t] = u[t] @ Kin + bias has no time dependence
    and is tiny; computed on the host, shipped pre-swizzled into the
    exact per-chunk SBUF layout so the per-iteration DMA is one fully
    contiguous block.
  - The time scan is the serial bottleneck: per step a [1024]x[1024,1024]
    matvec on TensorE as 64 LDWEIGHTS+MATMUL pairs of [128,128]x[128,1].
    Measured on HW: the pair cost (~41 ns) is pure NX instruction-issue
    overhead, independent of weight dtype (bf16 == fp8), so the step
    floor is 64 pairs x 41 ns = 2.6 us and weights stay bf16 for
    accuracy.  Leaky a is folded in: W' = a*W; state kept fp32 via the
    rescaled recurrence h'[t] = (1-a)*h'[t-1] + tanh(W' h'[t-1] + c[t]);
    the output is a*h'.
  - Per step the matmuls are phase-ordered (contraction tiles 0-3 for all
    output tiles, then finish output tiles 0-3, then 4-7) so ScalarE/
    VectorE process the first half of the new state while TensorE
    finishes the second half, and the next step's matmuls (which need
    only the first half as contraction input) start immediately.
  - tanh on ScalarE, z+c add and leaky blends on VectorE; the bf16 copy
    of the new state is written first to unblock TensorE.
  - Output states are staged in SBUF and DMAd per 32-step chunk in the
    SBUF-native layout; the host inverts the layout after gathering.
"""

import numpy as np
import ml_dtypes

import concourse.bacc as bacc
import concourse.tile as tile
import concourse.mybir as mybir
from concourse.bass import ds
from concourse.bass_utils import run_bass_kernel_spmd

F32 = mybir.dt.float32
BF16 = mybir.dt.bfloat16

UNITS = 1024
IN = 64
KT = 8  # contraction tiles (1024/128)
MT = 8  # output-unit tiles (1024/128)
P = 128

LEAKY = np.float32(0.9)
ONE_MINUS_LEAKY = float(np.float32(1.0) - np.float32(0.9))

N_CORES = 8
N_MODULES = 4
BURN = 64  # burn-in steps for the second-half cores (~4x measured convergence)


def build_nc(T_local: int, unroll: int):
    """Build the single-core SPMD Bass program for one reservoir module
    scanning T_local steps."""
    assert T_local % unroll == 0 and unroll % 2 == 0
    nchunk = T_local // unroll
    nc = bacc.Bacc("TRN2", debug=False)

    wT = nc.dram_tensor("wT", [UNITS, UNITS], BF16, kind="ExternalInput")
    # c pre-swizzled on host: c_in[chunk, p, s, j] = c[chunk*unroll+s, j*128+p]
    c_in = nc.dram_tensor("c_in", [nchunk, P, unroll, MT], F32, kind="ExternalInput")
    # output in SBUF-native layout: hs[chunk, p, s, j] = h[chunk*unroll+s, j*128+p]
    # (fp32: a bf16 output variant hit NRT_EXEC_UNIT_UNRECOVERABLE on HW)
    hs = nc.dram_tensor("hs", [nchunk, P, unroll, MT], F32, kind="ExternalOutput")

    with tile.TileContext(nc) as tc:
        with (
            tc.tile_pool(name="const", bufs=1) as const_pool,
            tc.tile_pool(name="cin", bufs=2) as cin_pool,
            tc.tile_pool(name="hout", bufs=2) as hout_pool,
            tc.tile_pool(name="work", bufs=2) as work_pool,
            tc.tile_pool(name="zpsum", bufs=2, space="PSUM") as zpsum_pool,
        ):
            # weights: w_sb[p, k, m, c] = W'[k*128+p, m*128+c]
            w_sb = const_pool.tile([P, KT, MT, P], BF16)
            nc.sync.dma_start(
                w_sb[:], wT[:, :].rearrange("(k p) (m c) -> p k m c", p=P, c=P)
            )

            # persistent scan state (ping-pong on dim 1 by step parity)
            hstate = const_pool.tile([P, 2, MT], F32)  # h' fp32 master
            h16 = const_pool.tile([P, 2, MT], BF16)  # bf16 copy for PE rhs
            nc.vector.memset(hstate[:, 1, :], 0.0)
            nc.vector.memset(h16[:, 1, :], 0.0)

            c_v = c_in[:, :, :, :].rearrange("c p s j -> p c s j")
            hs_v = hs[:, :, :, :].rearrange("c p s j -> p c s j")

            with tc.For_i(
                0,
                nchunk,
                1,
                hint_engines=(mybir.EngineType.PE, mybir.EngineType.Activation),
            ) as iv:
                cchunk = cin_pool.tile([P, unroll, MT], F32, tag="cchunk")
                nc.sync.dma_start(cchunk[:], c_v[:, ds(iv, 1), :, :])
                hstage = hout_pool.tile([P, unroll, MT], F32, tag="hstage")

                for s in range(unroll):
                    cur = s % 2
                    prev = 1 - cur
                    zA = zpsum_pool.tile([P, 4], F32, tag="zA")
                    zB = zpsum_pool.tile([P, 4], F32, tag="zB")

                    def mm(k, m, start, stop):
                        zt = zA if m < 4 else zB
                        nc.tensor.matmul(
                            zt[:, (m % 4) : (m % 4) + 1],
                            w_sb[:, k, m, :],
                            h16[:, prev, k : k + 1],
                            start=start,
                            stop=stop,
                        )

                    # phase 1: contraction tiles 0-3 (only needs half A of
                    # h16, which the previous step produced early)
                    for k in range(4):
                        for m in range(MT):
                            mm(k, m, start=(k == 0 and m % 4 == 0), stop=False)
                    # phase 2a: finish z columns 0-3 so ScalarE can start
                    for m in range(4):
                        for k in range(4, 8):
                            mm(k, m, start=False, stop=(k == 7 and m == 3))
                    # phase 2b: finish z columns 4-7
                    for m in range(4, 8):
                        for k in range(4, 8):
                            mm(k, m, start=False, stop=(k == 7 and m == 7))

                    zc = work_pool.tile([P, MT], F32, tag="zc")
                    o32 = work_pool.tile([P, MT], F32, tag="o32")
                    for (lo, hi), zt in (((0, 4), zA), ((4, 8), zB)):
                        # zc = z + c[t]
                        nc.vector.tensor_add(
                            zc[:, lo:hi], zt[:, 0:4], cchunk[:, s, lo:hi]
                        )
                        # o = tanh(zc)
                        nc.scalar.activation(
                            o32[:, lo:hi],
                            zc[:, lo:hi],
                            mybir.ActivationFunctionType.Tanh,
                        )
                        # critical-path first: bf16 state for the next matmuls
                        nc.vector.scalar_tensor_tensor(
                            out=h16[:, cur, lo:hi],
                            in0=hstate[:, prev, lo:hi],
                            scalar=ONE_MINUS_LEAKY,
                            in1=o32[:, lo:hi],
                            op0=mybir.AluOpType.mult,
                            op1=mybir.AluOpType.add,
                        )
                        # fp32 master state (off critical path)
                        nc.vector.scalar_tensor_tensor(
                            out=hstate[:, cur, lo:hi],
                            in0=hstate[:, prev, lo:hi],
                            scalar=ONE_MINUS_LEAKY,
                            in1=o32[:, lo:hi],
                            op0=mybir.AluOpType.mult,
                            op1=mybir.AluOpType.add,
                        )
                    # output h[t] = a * h'[t]
                    nc.vector.tensor_scalar_mul(
                        hstage[:, s, :], hstate[:, cur, :], float(LEAKY)
                    )

                nc.sync.dma_start(hs_v[:, ds(iv, 1), :, :], hstage[:])

    nc.compile()
    return nc


def _t_local(T: int, unroll: int) -> int:
    assert T % 2 == 0
    t = T // 2 + BURN
    # round up to a whole number of chunks
    t = ((t + unroll - 1) // unroll) * unroll
    return t


def _prep_in_maps(u, kernel, rec_kernel, bias, T, unroll):
    """Per-core inputs: core c runs module c%4; cores 0-3 scan the window
    [0, T_local), cores 4-7 the window [T - T_local, T)."""
    T_local = _t_local(T, unroll)
    nchunk = T_local // unroll
    u0 = np.asarray(u[0], dtype=np.float32)  # [T, 64]
    in_maps = []
    for core in range(N_CORES):
        m = core % N_MODULES
        half = core // N_MODULES
        t0 = 0 if half == 0 else T - T_local
        wT = np.ascontiguousarray(
            (np.asarray(rec_kernel[m], dtype=np.float32) * LEAKY).astype(
                ml_dtypes.bfloat16
            )
        )
        # c[t, u] = u[t] @ Kin + bias  (fp32, host)
        c = u0[t0 : t0 + T_local] @ np.asarray(kernel[m], dtype=np.float32) + (
            np.asarray(bias[m], dtype=np.float32)
        )
        # -> c_in[chunk, p, s, j]
        c_sw = np.ascontiguousarray(
            c.reshape(nchunk, unroll, MT, P).transpose(0, 3, 1, 2)
        )
        in_maps.append({"wT": wT, "c_in": c_sw})
    return in_maps


def _unswizzle(hs_dev, T_local, unroll):
    # hs_dev[chunk, p, s, j] -> [T_local, 1024] with unit u = j*128+p
    return np.ascontiguousarray(
        hs_dev.astype(np.float32).transpose(0, 2, 3, 1).reshape(T_local, UNITS)
    )


def _assemble(per_core_hs, T, unroll):
    """per_core_hs: list of 8 arrays [nchunk, P, unroll, MT] -> [1, T, 4096]."""
    T_local = _t_local(T, unroll)
    outs = []
    for m in range(N_MODULES):
        first = _unswizzle(per_core_hs[m], T_local, unroll)[: T // 2]
        second = _unswizzle(per_core_hs[m + N_MODULES], T_local, unroll)[
            T_local - (T - T // 2) :
        ]
        outs.append(np.concatenate([first, second], axis=0))
    out = np.concatenate(outs, axis=1)  # [T, 4096]
    return out[None].astype(np.float32)


_NC_CACHE = {}


def run(u, kernel, rec_kernel, bias, unroll=64, trace=False):
    T = u.shape[1]
    T_local = _t_local(T, unroll)
    key = (T_local, unroll)
    if key not in _NC_CACHE:
        _NC_CACHE[key] = build_nc(T_local, unroll)
    nc = _NC_CACHE[key]
    in_maps = _prep_in_maps(u, kernel, rec_kernel, bias, T, unroll)
    res = run_bass_kernel_spmd(
        nc, in_maps, core_ids=list(range(N_CORES)), trace=trace
    )
    out = _assemble([res.results[c]["hs"] for c in range(N_CORES)], T, unroll)
    return out, res


def kernel(u, kernel, rec_kernel, bias):
    out, _ = run(u, kernel, rec_kernel, bias)
    return out
